# revision 1
# baseline (speedup 1.0000x reference)
"""CenterLoss kernel for Trainium2 (Bass/Tile), data-parallel over 8 NeuronCores.

loss = 0.5 * sum_i ||x_i - centers[targets_i]||^2

The reference materializes the full [N, C] distance matrix and gathers one
entry per row; here we gather only the target center rows (indirect DMA) and
do a fused subtract / square-accumulate, so the kernel is memory-bound on
~4 MB of HBM traffic per core instead of a 69 GFLOP matmul.

Sharding: inputs/targets split along batch N across 8 cores (512 rows each),
centers replicated. Each core partition-reduces its partials on the (idle)
PE and returns a handful of scalars; the host sums them and scales by 0.5.
"""

import numpy as np

import concourse.bacc as bacc
import concourse.bass as bass
import concourse.tile as tile
from concourse import mybir
from concourse.bass_utils import run_bass_kernel_spmd

N, C, D = 4096, 8192, 1024
N_CORES = 8
ROWS = N // N_CORES  # 512 rows per core
P = 128              # SBUF partitions
CHUNKS = ROWS // P   # 4 chunks of 128 rows

# Stashed BassKernelResults from the most recent kernel() call (for profiling).
LAST_RESULTS = None
_NC_CACHE = None


def _build_bass():
    nc = bacc.Bacc("TRN2", target_bir_lowering=False)
    x = nc.dram_tensor("x", [ROWS, D], mybir.dt.float32, kind="ExternalInput")
    idx = nc.dram_tensor("idx", [P, CHUNKS], mybir.dt.int32, kind="ExternalInput")
    centers = nc.dram_tensor("centers", [C, D], mybir.dt.float32, kind="ExternalInput")
    NACC = CHUNKS + 1  # chunks 0-2 full width; chunk 3 in two half-width cols
    out = nc.dram_tensor("out", [1, NACC], mybir.dt.float32, kind="ExternalOutput")

    with tile.TileContext(nc) as tc:
        with (
            tc.tile_pool(name="io", bufs=1) as io,
            tc.tile_pool(name="cpool", bufs=CHUNKS) as cp,
            tc.tile_pool(name="psum", bufs=1, space="PSUM") as pp,
            tc.tile_pool(name="small", bufs=1) as small,
        ):
            # idx is the first DMA on the Sync ring — on quiet fabric it
            # completes in ~2.1 us and ungates the gather descriptor-gen.
            idx_sb = small.tile([P, CHUNKS], mybir.dt.int32)
            nc.sync.dma_start(idx_sb[:], idx[:, :])
            # Row r of the shard lives at partition p = r // CHUNKS, chunk
            # t = r % CHUNKS, so each partition's 4 rows are 16 KB contiguous
            # in DRAM. Two 1 MB DMAs (separate tiles) so chunks 0-1 start
            # computing without waiting on chunks 2-3's data.
            x_dram_halves = x.rearrange("(p g u) d -> p g (u d)", p=P, g=2)
            x_sb = []
            for g in range(2):
                xg = io.tile([P, 2 * D], mybir.dt.float32, tag=f"x{g}")
                nc.sync.dma_start(xg[:], x_dram_halves[:, g, :])
                x_sb.append(xg)
            ones = small.tile([P, 1], mybir.dt.float32)
            nc.vector.memset(ones[:], 1.0)
            # Dummy activation to pull the ACT function-table load off the
            # critical path (it otherwise lands right before the first real
            # ACTIVATE and delays the whole chain by ~1.3 us).
            warm = small.tile([1, 1], mybir.dt.float32)
            nc.scalar.activation(
                out=warm[:], in_=ones[0:1, :],
                func=mybir.ActivationFunctionType.Square,
            )
            acc = small.tile([P, NACC], mybir.dt.float32)
            for t in range(CHUNKS):
                ct = cp.tile([P, D], mybir.dt.float32, tag="c")
                nc.gpsimd.indirect_dma_start(
                    out=ct[:],
                    out_offset=None,
                    in_=centers[:, :],
                    in_offset=bass.IndirectOffsetOnAxis(
                        ap=idx_sb[:, t : t + 1], axis=0
                    ),
                )
                xg = x_sb[t // 2]
                xoff = (t % 2) * D
                if t < CHUNKS - 1:
                    # d = x - c (in place over the gathered centers)
                    nc.vector.tensor_sub(ct[:], xg[:, xoff : xoff + D], ct[:])
                    # acc col = sum_d d^2 (ACT: fused square + row-sum)
                    nc.scalar.activation(
                        out=ct[:],
                        in_=ct[:],
                        func=mybir.ActivationFunctionType.Square,
                        accum_out=acc[:, t : t + 1],
                    )
                else:
                    # Last chunk in half-width slices to shorten the final
                    # gather -> subtract -> square serial chain.
                    HD = D // 2
                    for h in range(2):
                        cs, ce = h * HD, (h + 1) * HD
                        nc.vector.tensor_sub(
                            ct[:, cs:ce], xg[:, xoff + cs : xoff + ce], ct[:, cs:ce]
                        )
                        nc.scalar.activation(
                            out=ct[:, cs:ce],
                            in_=ct[:, cs:ce],
                            func=mybir.ActivationFunctionType.Square,
                            accum_out=acc[:, t + h : t + h + 1],
                        )
            # Partition-reduce on the (idle) PE: ones^T @ acc-cols. Chunks
            # 0-2 are reduced and shipped while chunk 3 is still computing;
            # each output DMA is a single small descriptor so its HBM
            # write-ack flush is one engine instead of sixteen.
            psum_a = pp.tile([1, CHUNKS - 1], mybir.dt.float32, tag="pa")
            nc.tensor.matmul(
                psum_a[:], lhsT=ones[:], rhs=acc[:, : CHUNKS - 1],
                start=True, stop=True,
            )
            res_a = small.tile([1, CHUNKS - 1], mybir.dt.float32)
            nc.vector.tensor_copy(res_a[:], psum_a[:])
            nc.sync.dma_start(out[:, : CHUNKS - 1], res_a[:])
            psum_b = pp.tile([1, 2], mybir.dt.float32, tag="pb")
            nc.tensor.matmul(
                psum_b[:], lhsT=ones[:], rhs=acc[:, CHUNKS - 1 :],
                start=True, stop=True,
            )
            res_b = small.tile([1, 2], mybir.dt.float32)
            nc.vector.tensor_copy(res_b[:], psum_b[:])
            nc.sync.dma_start(out[:, CHUNKS - 1 :], res_b[:])
    nc.finalize()
    return nc


def _get_nc():
    global _NC_CACHE
    if _NC_CACHE is None:
        _NC_CACHE = _build_bass()
    return _NC_CACHE


def kernel(inputs, targets, centers):
    global LAST_RESULTS
    x = np.ascontiguousarray(np.asarray(inputs, dtype=np.float32))
    tgt = np.asarray(targets).astype(np.int32)
    cen = np.ascontiguousarray(np.asarray(centers, dtype=np.float32))
    assert x.shape == (N, D) and cen.shape == (C, D) and tgt.shape == (N,)

    nc = _get_nc()
    in_maps = []
    for c in range(N_CORES):
        xs = np.ascontiguousarray(x[c * ROWS : (c + 1) * ROWS])
        # idx[p, t] = target of shard row p*CHUNKS + t
        idxs = np.ascontiguousarray(tgt[c * ROWS : (c + 1) * ROWS].reshape(P, CHUNKS))
        in_maps.append({"x": xs, "idx": idxs, "centers": cen})

    res = run_bass_kernel_spmd(nc, in_maps, core_ids=list(range(N_CORES)))
    LAST_RESULTS = res

    total = 0.0
    for r in res.results:
        total += float(r["out"].astype(np.float64).sum())
    return np.array(0.5 * total, dtype=np.float32)

